# revision 18
# baseline (speedup 1.0000x reference)
"""Trainium2 Bass kernel for nn_BatchDropTop (topk row masking).

Reference math: per sample b, act = sum_c x[b,c,:,:]^2  -> [H,W]; L2-normalize
over flattened (H,W) (a positive per-sample scale -- cannot change any
ordering, so it is skipped); row score = max_w act -> [H]; drop (zero) the
rh=8 rows with the largest score; out = x * row_mask.

Kernel strategy (pure data parallel, batch 64 -> 8 samples on each of 8
cores; per core, per sample):
  - DMA x[s] (2048x24x8 f32, 1.5 MB) into SBUF as [128p, 16k, 192hw]
    (partition p holds channels 16p..16p+15; contiguous 12KB per partition).
    Loads alternate between the sync and scalar HWDGE rings, stores ride
    gpsimd/scalar -- one ring tops out around 260 GB/s and loads sharing a
    ring with stores FIFO-block behind them.
  - ACT: square elementwise (two halves, so PE starts early).
  - PE: 8 accumulating N=384 matmuls with a ones[128,1] stationary vector
    reduce the channel dim -> two partial sums [1, 2, 192] in PSUM, folded
    by one strided DVE reduce.
  - DVE: rowmax[1,24] = max over w; top8 = vector.max (8 largest, desc);
    mask[1,24,8] = (rowmax < top8[7]) as 1.0/0.0, with the compare input
    broadcast over w.  (Exactly the top-8 rows get 0; validated tie-free
    on the real inputs with 4.4e-5 min rel gap -- fp32 accumulation is
    required, bf16/tf32 noise would flip borderline rows.)
  - PE ones[1,128] K=1 matmul broadcasts the mask to [128,192] PSUM.
  - DVE: y = x * mask (mask AP broadcast over the chunk dim), DMA out in
    half-sample units.

Everything is read from HBM once and written once: 25.2 MB per core
~= 70 us at the ~358 GB/s per-core HBM roofline; measured ~78-82 us per
core (NTFF) incl. ~3 us startup and ~9 us Tile drain, with ACT/PE/DVE
(25-56 us each) hidden under the DMA stream by the Tile scheduler.
"""

import sys

import numpy as np

for _p in ("/opt/trn_rl_repo", "/root/.axon_site/_ro/trn_rl_repo"):
    if _p not in sys.path:
        sys.path.append(_p)

B, C, H, W = 64, 2048, 24, 8
N_CORES = 8
BS = B // N_CORES  # samples per core
P = 128            # SBUF partitions
KC = C // P        # channel chunks per sample
HW = H * W
RH = 8             # rows to drop == round(0.33 * 24)

_cache = {}


def _build_nc():
    from concourse import bacc, mybir, tile

    f32 = mybir.dt.float32
    nc = bacc.Bacc("TRN2", target_bir_lowering=False, debug=False,
                   num_devices=N_CORES)
    x_in = nc.dram_tensor("x", [BS, C, H, W], f32, kind="ExternalInput")
    y_out = nc.dram_tensor("out", [BS, C, H, W], f32, kind="ExternalOutput")

    with tile.TileContext(nc) as tc:
        with (
            tc.tile_pool(name="xp", bufs=BS) as xp,
            tc.tile_pool(name="sq", bufs=3) as sqp,
            tc.tile_pool(name="yp", bufs=3) as yp,
            tc.tile_pool(name="const", bufs=1) as constp,
            tc.tile_pool(name="tmp", bufs=3) as tmpp,
            tc.tile_pool(name="small", bufs=BS) as smallp,
            tc.tile_pool(name="psA", bufs=3, space="PSUM") as psA,
            tc.tile_pool(name="psB", bufs=3, space="PSUM") as psB,
        ):
            ones_col = constp.tile([P, 1], f32)  # stationary K=128 reducer
            nc.vector.memset(ones_col[:], 1.0)
            ones_row = constp.tile([1, P], f32)  # stationary K=1 broadcaster
            nc.vector.memset(ones_row[:], 1.0)

            KH = KC // 2
            # Emit ALL loads first: with a full set of x buffers every load
            # enqueues immediately, and both HWDGE rings drain them densely.
            # Program order also guarantees the loads sit ahead of any store
            # on scalar's ring, so stores never FIFO-block a load.
            xts = []
            for s in range(BS):
                ld_eng = nc.sync if s % 2 == 0 else nc.scalar
                xt = xp.tile([P, KC, HW], f32, tag="x")
                x_dram = x_in[s].rearrange("(p k) h w -> p k (h w)", p=P)
                if s == 0:
                    # Sample 0 gates the whole store stream: halve its load
                    # latency by splitting it across both HWDGE rings.
                    nc.sync.dma_start(out=xt[:, :KH, :], in_=x_dram[:, :KH, :])
                    nc.scalar.dma_start(out=xt[:, KH:, :], in_=x_dram[:, KH:, :])
                else:
                    ld_eng.dma_start(out=xt[:], in_=x_dram[:])
                xts.append(xt)

            for s in range(BS):
                # Stores alternate gpsimd/scalar so the tail of the store
                # stream always has two rings available.
                st_eng = nc.gpsimd if s % 2 == 0 else nc.scalar
                xt = xts[s]

                # Square in two halves so PE can start reducing half A
                # while ACT squares half B.
                xsq = sqp.tile([P, KC, HW], f32, tag="sq")
                nc.scalar.square(xsq[:, :KH, :], xt[:, :KH, :])
                nc.scalar.square(xsq[:, KH:, :], xt[:, KH:, :])

                # Channel reduction, split across engines: the fp32 PE
                # matmul runs dual-pass (4 cyc/col) and is the late-phase
                # pacer, so the idle gpsimd pre-folds the last 4 chunks
                # (12..15) with 3 adds, cutting PE's streamed columns ~19%.
                tA = tmpp.tile([P, HW], f32, tag="tA")
                nc.gpsimd.tensor_tensor(tA[:], xsq[:, KC - 4, :],
                                        xsq[:, KC - 3, :],
                                        op=mybir.AluOpType.add)
                tB = tmpp.tile([P, HW], f32, tag="tB")
                nc.gpsimd.tensor_tensor(tB[:], xsq[:, KC - 2, :],
                                        xsq[:, KC - 1, :],
                                        op=mybir.AluOpType.add)
                tC = tmpp.tile([P, HW], f32, tag="tC")
                nc.gpsimd.tensor_tensor(tC[:], tA[:], tB[:],
                                        op=mybir.AluOpType.add)

                # PE: 6 accumulating N=384 matmuls over chunks 0..11, plus
                # one N=192 matmul folding in gpsimd's partial sum.
                act2 = psA.tile([1, 2, HW], f32, tag="act")
                for j in range(KC // 2 - 2):
                    nc.tensor.matmul(
                        act2[:], ones_col[:], xsq[:, 2 * j:2 * j + 2, :],
                        start=(j == 0), stop=False,
                    )
                nc.tensor.matmul(act2[:, 0, :], ones_col[:], tC[:],
                                 start=False, stop=True)
                act = smallp.tile([1, HW], f32, tag="actsb")
                nc.vector.tensor_reduce(
                    act[:], act2[:].transpose([0, 2, 1]),
                    axis=mybir.AxisListType.X, op=mybir.AluOpType.add,
                )

                rowmax = smallp.tile([1, H], f32, tag="rowmax")
                nc.vector.tensor_reduce(
                    rowmax[:],
                    act[:].rearrange("p (h w) -> p h w", h=H),
                    axis=mybir.AxisListType.X,
                    op=mybir.AluOpType.max,
                )
                top8 = smallp.tile([1, RH], f32, tag="top8")
                nc.vector.max(top8[:], rowmax[:])
                # mask over (h, w) in one shot: compare rowmax (broadcast
                # over w) against the 8th-largest value.
                maskhw = smallp.tile([1, HW], f32, tag="maskhw")
                nc.vector.tensor_single_scalar(
                    maskhw[:].rearrange("p (h w) -> p h w", h=H),
                    rowmax[:].unsqueeze(2).broadcast_to([1, H, W]),
                    top8[0:1, RH - 1:RH],
                    mybir.AluOpType.is_lt,
                )

                mb = psB.tile([P, HW], f32, tag="mb")
                nc.tensor.matmul(mb[:], ones_row[:], maskhw[:],
                                 start=True, stop=True)

                # Multiply + store in half-sample units: finer pipelining
                # and a shorter end-of-kernel tail.
                yt = yp.tile([P, KC, HW], f32, tag="y")
                y_dram = y_out[s].rearrange("(p k) h w -> p k (h w)", p=P)
                for half in range(2):
                    ksl = slice(half * KH, (half + 1) * KH)
                    nc.vector.tensor_tensor(
                        yt[:, ksl, :], xt[:, ksl, :],
                        mb[:].unsqueeze(1).broadcast_to([P, KH, HW]),
                        op=mybir.AluOpType.mult,
                    )
                    st_eng.dma_start(out=y_dram[:, ksl, :], in_=yt[:, ksl, :])

    nc.compile()
    return nc


def get_nc():
    if "nc" not in _cache:
        _cache["nc"] = _build_nc()
    return _cache["nc"]


def kernel(x):
    from concourse.bass_utils import run_bass_kernel_spmd

    x = np.ascontiguousarray(np.asarray(x, dtype=np.float32))
    assert x.shape == (B, C, H, W), x.shape
    nc = get_nc()
    in_maps = [{"x": x[i * BS:(i + 1) * BS]} for i in range(N_CORES)]
    res = run_bass_kernel_spmd(nc, in_maps, list(range(N_CORES)))
    return np.concatenate(
        [res.results[i]["out"] for i in range(N_CORES)], axis=0
    )
